# revision 1
# baseline (speedup 1.0000x reference)
"""Channel-wise row attention kernel for Trainium2 (8 NeuronCores).

Reference computation (per (n, w) slab, with qp = q[n,:,:,w].T etc. of shape (H, C)):
    attn = softmax(qp @ kp.T / sqrt(C), axis=-1);  out_slab = (attn @ vp).T  # (C, H)

Sharding: (n, w-quarter) across 8 cores -> each core owns 64 independent slabs.
Host pre-permutes inputs so each core receives contiguous per-slab operands:
    q_dev, k_dev: (64, C, H)   (the natural matmul layout: contraction dim C on partitions)
    v_dev:        (64, H, C)   (pre-transposed so stage-2 lhsT needs no on-device transpose)
Device (per slab, partition-dim softmax to avoid any on-chip transposes):
    S^T[g,h] = sum_c k[c,g] q[c,h]                 4 matmuls (lhsT=k-block, rhs=q)
    E = exp(S^T / sqrt(C))                         ScalarE, PSUM->SBUF
    colsum[*,h] = ones(128,128)^T @ E (4 acc.)     matmul => broadcast column sums
    R = 1/colsum                                   VectorE reciprocal
    O[c,h] = sum_g v^T-block @ E (4 acc.)          matmuls
    out = O * R                                    VectorE (also evicts PSUM->SBUF)
All matmuls run as float32r (fp32 storage, full-rate PE streaming).
"""

import numpy as np
from contextlib import ExitStack

import concourse.bass as bass
import concourse.bacc as bacc
import concourse.tile as tile
import concourse.mybir as mybir
from concourse.bass_utils import run_bass_kernel_spmd

N, C, H, W = 2, 128, 512, 256
NCORES = 8
WQ = 4                 # w-quarters per n
WPC = W // WQ          # 64 slabs per core
GT = H // 128          # 4 g-tiles per slab
SCALE = float(1.0 / np.sqrt(np.float32(C)))
F32 = mybir.dt.float32
F32R = mybir.dt.float32r


def _body(
    ctx: ExitStack,
    tc: tile.TileContext,
    qd,
    kd,
    vd,
    od,
    n_slabs: int,
    group: int,
    repeat: int = 1,
):
    nc = tc.nc
    import os
    in_bufs = int(os.environ.get("KB_IN_BUFS", "3"))
    out_bufs = int(os.environ.get("KB_OUT_BUFS", "2"))
    e_bufs = int(os.environ.get("KB_E_BUFS", "2"))
    ps_s_bufs = int(os.environ.get("KB_PS_S", "4"))
    ps_cs_bufs = int(os.environ.get("KB_PS_CS", "2"))
    ps_o_bufs = int(os.environ.get("KB_PS_O", "2"))
    const_pool = ctx.enter_context(tc.tile_pool(name="const", bufs=1))
    in_pool = ctx.enter_context(tc.tile_pool(name="inp", bufs=in_bufs))
    e_pool = ctx.enter_context(tc.tile_pool(name="epool", bufs=e_bufs))
    r_pool = ctx.enter_context(tc.tile_pool(name="rpool", bufs=2))
    out_pool = ctx.enter_context(tc.tile_pool(name="outp", bufs=out_bufs))
    ps_s = ctx.enter_context(tc.tile_pool(name="ps_s", bufs=ps_s_bufs, space="PSUM"))
    ps_cs = ctx.enter_context(tc.tile_pool(name="ps_cs", bufs=ps_cs_bufs, space="PSUM"))
    ps_o = ctx.enter_context(tc.tile_pool(name="ps_o", bufs=ps_o_bufs, space="PSUM"))

    ones_f32 = const_pool.tile([128, 128], F32, name="ones_f32")
    nc.vector.memset(ones_f32, 1.0)
    ones_t = const_pool.tile([128, 128], F32R, name="ones_t")
    nc.scalar.activation(ones_t, ones_f32, mybir.ActivationFunctionType.Copy)

    n_groups = n_slabs // group
    for gi in range(n_groups * repeat):
        gi = gi % n_groups
        w0 = gi * group
        # Group DMAs: (c, s, h) tiles; per-(c,s) runs of H are contiguous in DRAM.
        q_g = in_pool.tile([C, group, H], F32R, tag="q", name="q_g")
        nc.sync.dma_start(out=q_g, in_=qd[w0 : w0 + group].rearrange("s c h -> c s h"))
        k_g = in_pool.tile([C, group, H], F32R, tag="k", name="k_g")
        _kdma = nc.scalar if os.environ.get("KB_SPLIT_DMA", "0") == "1" else nc.sync
        _kdma.dma_start(out=k_g, in_=kd[w0 : w0 + group].rearrange("s c h -> c s h"))
        # v: (s, (t p), c) -> partitions p, free (s, t, c); c-runs (512B) contiguous.
        v_g = in_pool.tile([128, group, GT, C], F32R, tag="v", name="v_g")
        nc.sync.dma_start(
            out=v_g, in_=vd[w0 : w0 + group].rearrange("s (t p) c -> p s t c", p=128)
        )
        out_g = out_pool.tile([C, group, H], F32, tag="out", name="out_g")

        for j in range(group):
            q_t = q_g[:, j, :]
            k_t = k_g[:, j, :]
            e_ts = []
            for t in range(GT):
                s_ps = ps_s.tile([128, H], F32, tag="s", name="s_ps")
                nc.tensor.matmul(
                    s_ps,
                    lhsT=k_t[:, t * 128 : (t + 1) * 128],
                    rhs=q_t,
                    start=True,
                    stop=True,
                )
                e_t = e_pool.tile([128, H], F32R, tag=f"e{t}", name="e_t")
                nc.scalar.activation(
                    e_t, s_ps, mybir.ActivationFunctionType.Exp, scale=SCALE
                )
                e_ts.append(e_t)

            cs_ps = ps_cs.tile([128, H], F32, tag="cs", name="cs_ps")
            if os.environ.get("KB_CS_FOLD", "0") == "1":
                ea = e_pool.tile([128, H], F32R, tag="ea", name="ea")
                nc.vector.tensor_add(ea, e_ts[0], e_ts[1])
                eb = e_pool.tile([128, H], F32R, tag="eb", name="eb")
                nc.vector.tensor_add(eb, e_ts[2], e_ts[3])
                ec = e_pool.tile([128, H], F32R, tag="ec", name="ec")
                nc.vector.tensor_add(ec, ea, eb)
                nc.tensor.matmul(cs_ps, lhsT=ones_t, rhs=ec, start=True, stop=True)
            else:
                for t in range(GT):
                    nc.tensor.matmul(
                        cs_ps,
                        lhsT=ones_t,
                        rhs=e_ts[t],
                        start=(t == 0),
                        stop=(t == GT - 1),
                    )
            r_t = r_pool.tile([128, H], F32, tag="r", name="r_t")
            nc.vector.reciprocal(r_t, cs_ps)

            o_ps = ps_o.tile([128, H], F32, tag="o", name="o_ps")
            for t in range(GT):
                nc.tensor.matmul(
                    o_ps,
                    lhsT=v_g[:, j, t, :],
                    rhs=e_ts[t],
                    start=(t == 0),
                    stop=(t == GT - 1),
                )
            nc.vector.tensor_mul(out_g[:, j, :], o_ps, r_t)

        nc.gpsimd.dma_start(out=od[w0 : w0 + group].rearrange("s c h -> c s h"), in_=out_g)


def build_nc(
    n_slabs: int = WPC, group: int = 8, repeat: int = 1, timing_mode: bool = False
) -> bass.Bass:
    """timing_mode: q/k/v become Internal DRAM scratch (contents irrelevant for
    timing; engine timing is data-independent) so the only external input is a
    small seed tensor -- removes host->device transfer from wall-clock."""
    nc = bacc.Bacc("TRN2", target_bir_lowering=False, debug=False)
    kind = "Internal" if timing_mode else "ExternalInput"
    qd = nc.dram_tensor("qi" if timing_mode else "q", [n_slabs, C, H], F32R, kind=kind).ap()
    kd = nc.dram_tensor("ki" if timing_mode else "k", [n_slabs, C, H], F32R, kind=kind).ap()
    vd = nc.dram_tensor("vi" if timing_mode else "v", [n_slabs, H, C], F32R, kind=kind).ap()
    seed = osmall = None
    if timing_mode:
        od = nc.dram_tensor("oi", [n_slabs, C, H], F32, kind="Internal").ap()
        seed = nc.dram_tensor("seed", [128, 128], F32R, kind="ExternalInput").ap()
        osmall = nc.dram_tensor("osmall", [128, 128], F32, kind="ExternalOutput").ap()
    else:
        od = nc.dram_tensor("o", [n_slabs, C, H], F32, kind="ExternalOutput").ap()
    with tile.TileContext(nc) as tc, ExitStack() as ctx:
        if timing_mode:
            # Fill internal q/k/v fully with real (small) values: garbage fp32
            # would generate NaN/Inf runtime notifications that distort timing.
            sp = ctx.enter_context(tc.tile_pool(name="seedp", bufs=1))
            st = sp.tile([128, 128], F32R, name="st")
            nc.sync.dma_start(out=st, in_=seed)
            st_b = bass.AP(
                tensor=st.tensor,
                offset=st.offset,
                ap=[list(st.ap[0]), [0, GT], list(st.ap[-1])],
            )
            for s in range(n_slabs):
                nc.gpsimd.dma_start(
                    out=qd[s].rearrange("c (t f) -> c t f", f=128), in_=st_b
                )
                nc.gpsimd.dma_start(
                    out=kd[s].rearrange("c (t f) -> c t f", f=128), in_=st_b
                )
                nc.gpsimd.dma_start(
                    out=vd[s].rearrange("(t p) c -> p t c", p=128), in_=st_b
                )
            if repeat > 1:
                with tc.For_i(0, repeat, 1):
                    _body(ctx, tc, qd, kd, vd, od, n_slabs, group, 1)
            else:
                _body(ctx, tc, qd, kd, vd, od, n_slabs, group, 1)
            st2 = sp.tile([128, 128], F32, name="st2")
            nc.vector.memset(st2, 2.0)
            nc.sync.dma_start(out=osmall, in_=st2)
        else:
            _body(ctx, tc, qd, kd, vd, od, n_slabs, group, repeat)
    nc.compile()
    return nc


def shard_inputs(q: np.ndarray, k: np.ndarray, v: np.ndarray) -> list[dict]:
    """Host-side shard + permute: core i gets n = i // WQ, w in [64*(i%WQ), ...)."""
    in_maps = []
    for i in range(NCORES):
        n, wq = divmod(i, WQ)
        ws = slice(wq * WPC, (wq + 1) * WPC)
        # (C, H, W') -> (W', C, H)
        qs = np.ascontiguousarray(np.transpose(q[n, :, :, ws], (2, 0, 1)))
        ks = np.ascontiguousarray(np.transpose(k[n, :, :, ws], (2, 0, 1)))
        # v pre-transposed: (W', H, C)
        vs = np.ascontiguousarray(np.transpose(v[n, :, :, ws], (2, 1, 0)))
        in_maps.append({"q": qs, "k": ks, "v": vs})
    return in_maps


def unshard_output(results: list[dict]) -> np.ndarray:
    out = np.empty((N, C, H, W), dtype=np.float32)
    for i in range(NCORES):
        n, wq = divmod(i, WQ)
        ws = slice(wq * WPC, (wq + 1) * WPC)
        out[n, :, :, ws] = np.transpose(results[i]["o"], (1, 2, 0))
    return out


_NC_CACHE = {}


def kernel(q: np.ndarray, k: np.ndarray, v: np.ndarray, **run_kwargs) -> np.ndarray:
    q = np.asarray(q, dtype=np.float32)
    k = np.asarray(k, dtype=np.float32)
    v = np.asarray(v, dtype=np.float32)
    key = "default"
    if key not in _NC_CACHE:
        _NC_CACHE[key] = build_nc()
    nc = _NC_CACHE[key]
    in_maps = shard_inputs(q, k, v)
    res = run_bass_kernel_spmd(nc, in_maps, core_ids=list(range(NCORES)), **run_kwargs)
    out = unshard_output(res.results)
    if run_kwargs.get("trace"):
        kernel.last_result = res
    return out

